# revision 15
# baseline (speedup 1.0000x reference)
"""Trainium2 Bass kernel for nn_DenseAttention (feature-axis attention over a
huge batch), data-parallel over 8 NeuronCores.

Math restructure (per core, batch shard x_s of 32768 rows):
  scores = q.T @ k contracts over batch -> scores = Wq G Wk.T + rank-1 bias
  terms, with G = x.T x (feature Gram) and s = x.T 1. G|s accumulate in PSUM
  over 128-row tiles and need a single [128,130] fp32 AllReduce.
  The reference's flat reshape maps attn columns to per-tile output rows, so
  output collapses per 128-row tile c to  y_block = MvT.T @ V_cT + corr,
  V_cT = X_c.T @ Wo.T, with Mv = softmax_weights @ Wv and
  corr = (weights@bv) x (Wo@1) + bo.

All heavy matmuls run in bf16: keeps the PE HAM clock gate at K=8/8
(2.4 GHz; f32r matmuls do not register as PE-busy), enables fast weight
load, and halves HBM traffic (x streamed bf16, y stored bf16; softmax
weights verified stable: min score top-gap 2.3 vs bf16-G score err <1.5).
Schedule: G-only pass (gates the single AllReduce) -> V pass + smalls
overlap the collective -> pass 2 writes bf16 output on two DMA queues.
"""
import functools

import numpy as np

B = 262144
D = 128
NCORES = 8
BS = B // NCORES          # rows per core
NT = BS // 128            # 128-row tiles per core (256)
CHUNK = 16                # tiles per input DMA
P2B = 8                   # tiles per pass-2 output block (2 matmuls)
WARM = 8                  # bf16 warmup matmuls (lift HAM clock during DMA)
FILL = 64                 # bf16 filler matmuls bridging the AllReduce wait
                          # (keep the HAM clock gate open into pass 2)
ISQ = 1.0 / np.sqrt(128.0)


@functools.lru_cache(maxsize=1)
def _build():
    import concourse.bass as bass  # noqa: F401
    import concourse.tile as tile
    from concourse import bacc, mybir

    f32 = mybir.dt.float32
    bf16 = mybir.dt.bfloat16
    AF = mybir.ActivationFunctionType
    OP = mybir.AluOpType

    nc = bacc.Bacc("TRN2", target_bir_lowering=False, debug=False,
                   num_devices=NCORES)

    x = nc.dram_tensor("x", [D, NT, 130], bf16, kind="ExternalInput").ap()
    wot = nc.dram_tensor("wot", [D, D], bf16, kind="ExternalInput").ap()
    consts8 = nc.dram_tensor("consts8", [D, 8, D], f32,
                             kind="ExternalInput").ap()
    consts3 = nc.dram_tensor("consts3", [D, 3], f32,
                             kind="ExternalInput").ap()
    y = nc.dram_tensor("y", [D, NT, D], bf16, kind="ExternalOutput").ap()
    corr_out = nc.dram_tensor("corr_out", [D, D], f32,
                              kind="ExternalOutput").ap()

    with tile.TileContext(nc) as tc:
        with tc.tile_pool(name="const", bufs=1) as constp, \
             tc.tile_pool(name="xall", bufs=1) as xallp, \
             tc.tile_pool(name="vstore", bufs=1) as vstorep, \
             tc.tile_pool(name="small", bufs=1) as smallp, \
             tc.tile_pool(name="obp", bufs=6) as obp, \
             tc.tile_pool(name="gps", bufs=1, space="PSUM") as gps, \
             tc.tile_pool(name="vps", bufs=3, space="PSUM") as vps, \
             tc.tile_pool(name="p2ps", bufs=3, space="PSUM") as p2ps, \
             tc.tile_pool(name="sps", bufs=1, space="PSUM") as sps, \
             tc.tile_pool(name="dram", bufs=1, space="DRAM") as dramp:

            cin = dramp.tile([D, 130], f32)
            cout = dramp.tile([D, 130], f32)

            xall = xallp.tile([D, NT, 130], bf16)
            V_sb = vstorep.tile([D, NT * D], bf16)

            # input stream first: even chunks on the scalar queue (which has
            # nothing ahead of it), odd on sync; constants follow on sync
            # (not needed until pass 1b / smalls).
            for ch in range(NT // CHUNK):
                eng = nc.scalar if ch % 2 == 0 else nc.sync
                eng.dma_start(
                    xall[:, ch * CHUNK:(ch + 1) * CHUNK, :],
                    x[:, ch * CHUNK:(ch + 1) * CHUNK, :])

            wot_sb = constp.tile([D, D], bf16)
            nc.sync.dma_start(wot_sb[:], wot)
            c8 = constp.tile([D, 8, D], f32)
            nc.sync.dma_start(c8[:], consts8)
            c3 = constp.tile([D, 3], f32)
            nc.sync.dma_start(c3[:], consts3)
            wqt_sb = c8[:, 0, :]
            wkt_sb = c8[:, 1, :]
            wv_sb = c8[:, 2, :]
            id_sb = c8[:, 3, :]
            bqrep_sb = c8[:, 4, :]
            bkrep_sb = c8[:, 5, :]
            wsrep_sb = c8[:, 6, :]
            borep_sb = c8[:, 7, :]
            bqcol_sb = c3[:, 0:1]
            bvcol_sb = c3[:, 1:3]

            # warmup bf16 matmuls: lift the PE HAM clock gate while the
            # first input chunks stream in.
            wm = constp.tile([D, 512], bf16)
            nc.vector.memset(wm[:], 0.25)
            for i in range(WARM):
                wpsu = sps.tile([D, 512], f32, tag="sm", name=f"wm{i}")
                nc.tensor.matmul(wpsu[:], wm[:, 0:128], wm[:],
                                 start=True, stop=True)

            # ---------------- pass 1a: G|s accumulation ----------------
            g_ps = gps.tile([D, 130], f32, name="g")
            with nc.named_scope("pass1a"):
                for c in range(NT):
                    nc.tensor.matmul(
                        g_ps[:], xall[:, c, 0:128], xall[:, c, 0:130],
                        start=(c == 0), stop=(c == NT - 1))
            g_sb = smallp.tile([D, 130], f32)
            nc.vector.tensor_copy(g_sb[:], g_ps[:])
            nc.sync.dma_start(cin[:], g_sb[:])
            nc.gpsimd.collective_compute(
                "AllReduce", OP.add,
                replica_groups=[list(range(NCORES))],
                ins=[cin.opt()], outs=[cout.opt()])
            allr = smallp.tile([D, 130], f32)
            nc.sync.dma_start(allr[:], cout[:])

            # ---------------- pass 1b: V tiles (overlap the AllReduce) ----
            with nc.named_scope("pass1b"):
                for q in range(NT // 4):
                    v_ps = vps.tile([D, 4, D], f32, name="v")
                    for t4 in range(4):
                        c = q * 4 + t4
                        nc.tensor.matmul(v_ps[:, t4, :],
                                         xall[:, c, 0:128], wot_sb[:],
                                         start=True, stop=True)
                    dst = V_sb[:, q * 4 * D:(q + 1) * 4 * D]
                    if q % 2 == 0:
                        nc.scalar.activation(dst, v_ps[:], AF.Copy)
                    else:
                        nc.vector.tensor_copy(dst, v_ps[:])

            # filler matmuls: keep the PE busy (HAM at K=8/8) while the
            # AllReduce completes, so pass 2 starts at full clock.
            for i in range(FILL):
                fps = sps.tile([D, 512], f32, tag="sm", name=f"fl{i}")
                nc.tensor.matmul(fps[:], wm[:, 0:128], wm[:],
                                 start=True, stop=True)

            # ---------------- smalls: scores, softmax, MvT, corr ----------
            with nc.named_scope("smalls"):
                g2 = allr[:, 0:128]
                s_col = allr[:, 128:129]

                # T1T = G @ WqT + s x bq
                t1_ps = sps.tile([D, 512], f32, tag="sm", name="t1")
                nc.tensor.matmul(t1_ps[:, 0:128], g2, wqt_sb,
                                 start=True, stop=True)
                t1_sb = smallp.tile([D, D], f32)
                nc.vector.tensor_scalar(t1_sb[:], bqrep_sb, s_col, None,
                                        op0=OP.mult)
                nc.vector.tensor_tensor(t1_sb[:], t1_sb[:], t1_ps[:, 0:128],
                                        OP.add)

                # uT = Wq s + B bq   [h, 1]
                ut_ps = sps.tile([D, 512], f32, tag="sm", name="ut")
                nc.tensor.matmul(ut_ps[:, 0:2], wqt_sb, allr[:, 128:130],
                                 start=True, stop=True)
                ut_sb = smallp.tile([D, 1], f32)
                nc.vector.tensor_scalar(ut_sb[:], bqcol_sb, float(B), None,
                                        op0=OP.mult)
                nc.vector.tensor_tensor(ut_sb[:], ut_sb[:], ut_ps[:, 0:1],
                                        OP.add)

                # scores = T1T.T @ WkT + uT x bk
                sc_ps = sps.tile([D, 512], f32, tag="sm", name="sc")
                nc.tensor.matmul(sc_ps[:, 0:128], t1_sb[:], wkt_sb,
                                 start=True, stop=True)
                sc_sb = smallp.tile([D, D], f32)
                nc.vector.tensor_scalar(sc_sb[:], bkrep_sb, ut_sb[:, :], None,
                                        op0=OP.mult)
                nc.vector.tensor_tensor(sc_sb[:], sc_sb[:], sc_ps[:, 0:128],
                                        OP.add)

                # softmax over free dim with 1/sqrt(128) scaling
                mx = smallp.tile([D, 1], f32)
                nc.vector.reduce_max(mx[:], sc_sb[:], axis=mybir.AxisListType.X)
                mxn = smallp.tile([D, 1], f32)
                nc.vector.tensor_scalar(mxn[:], mx[:], -ISQ, None, op0=OP.mult)
                wts = smallp.tile([D, D], f32)
                rs = smallp.tile([D, 1], f32)
                nc.scalar.activation(wts[:], sc_sb[:], AF.Exp,
                                     bias=mxn[:, :], scale=ISQ, accum_out=rs[:])
                ri = smallp.tile([D, 1], f32)
                nc.vector.reciprocal(ri[:], rs[:])
                nc.vector.tensor_scalar(wts[:], wts[:], ri[:, :], None,
                                        op0=OP.mult)

                # weightsT, MvT, cc, corr
                wt_ps = sps.tile([D, 512], f32, tag="sm", name="wt")
                nc.tensor.transpose(wt_ps[:, 0:128], wts[:], id_sb)
                wtT_sb = smallp.tile([D, D], f32)
                nc.vector.tensor_copy(wtT_sb[:], wt_ps[:, 0:128])
                mvt_ps = sps.tile([D, 512], f32, tag="sm", name="mvt")
                nc.tensor.matmul(mvt_ps[:, 0:128], wv_sb, wtT_sb[:],
                                 start=True, stop=True)
                mvt_sb = smallp.tile([D, D], bf16)
                nc.vector.tensor_copy(mvt_sb[:], mvt_ps[:, 0:128])
                cc_ps = sps.tile([D, 512], f32, tag="sm", name="cc")
                nc.tensor.matmul(cc_ps[:, 0:2], wtT_sb[:], bvcol_sb,
                                 start=True, stop=True)
                cc_sb = smallp.tile([D, 1], f32)
                nc.vector.tensor_copy(cc_sb[:], cc_ps[:, 0:1])
                corr = smallp.tile([D, D], f32)
                nc.vector.tensor_scalar(corr[:], wsrep_sb, cc_sb[:, :], None,
                                        op0=OP.mult)
                nc.vector.tensor_tensor(corr[:], corr[:], borep_sb, OP.add)
                nc.sync.dma_start(corr_out, corr[:])

            # ---------------- pass 2 (corr is added host-side) -------------
            with nc.named_scope("pass2"):
                for blk in range(NT // P2B):
                    ob = obp.tile([D, P2B, D], bf16)
                    for h in range(2):
                        p2 = p2ps.tile([D, 4 * D], f32, name="p2")
                        off = (blk * P2B + h * 4) * D
                        nc.tensor.matmul(
                            p2[:], mvt_sb[:],
                            V_sb[:, off:off + 4 * D],
                            start=True, stop=True)
                        dst = ob[:, h * 4:(h + 1) * 4, :]
                        src = p2[:].rearrange("p (b o) -> p b o", b=4)
                        if h == 0:
                            nc.scalar.activation(dst, src, AF.Copy)
                        else:
                            nc.vector.tensor_copy(dst, src)
                    eng = nc.sync if blk % 2 == 0 else nc.gpsimd
                    eng.dma_start(y[:, blk * P2B:(blk + 1) * P2B, :], ob[:])

    nc.compile()
    return nc


def kernel(x, Wq, bq, Wk, bk, Wv, bv, Wo, bo):
    import ml_dtypes
    from concourse import bass_utils

    f = np.float32
    bf = ml_dtypes.bfloat16
    x = np.ascontiguousarray(np.asarray(x, f))
    Wq = np.asarray(Wq, f); bq = np.asarray(bq, f)
    Wk = np.asarray(Wk, f); bk = np.asarray(bk, f)
    Wv = np.asarray(Wv, f); bv = np.asarray(bv, f)
    Wo = np.asarray(Wo, f); bo = np.asarray(bo, f)

    consts8 = np.stack([
        Wq.T, Wk.T, Wv, np.eye(D, dtype=f),
        np.broadcast_to(bq, (D, D)), np.broadcast_to(bk, (D, D)),
        np.broadcast_to(Wo.sum(1), (D, D)), np.broadcast_to(bo, (D, D)),
    ], axis=1).astype(f)
    consts3 = np.stack([bq, bv, bv], axis=1).astype(f)
    shared = {
        "wot": np.ascontiguousarray(Wo.T.astype(bf)),
        "consts8": np.ascontiguousarray(consts8),
        "consts3": np.ascontiguousarray(consts3),
    }
    # [B, 128] -> per core [128 partitions(p), NT tiles(c), 130] bf16 with two
    # ones columns (row r of core s = tile c = (r - s*BS)//128, p = r%128)
    xb = x.astype(bf).reshape(NCORES, NT, 128, D).transpose(0, 2, 1, 3)
    xp = np.empty((NCORES, D, NT, 130), bf)
    xp[..., 0:128] = xb
    xp[..., 128:130] = np.asarray(1.0, bf)
    in_maps = [
        {"x": np.ascontiguousarray(xp[s]), **shared}
        for s in range(NCORES)
    ]

    nc = _build()
    res = bass_utils.run_bass_kernel_spmd(nc, in_maps,
                                          core_ids=list(range(NCORES)))
    kernel.last_result = res
    y = np.concatenate([res.results[s]["y"] for s in range(NCORES)], axis=1)
    y = y.astype(np.float32)
    y += res.results[0]["corr_out"][:, None, :]   # corr identical on all cores
    return np.ascontiguousarray(y.reshape(B, D))
